# revision 1
# baseline (speedup 1.0000x reference)
"""Trainium2 Bass kernel for nn_ActorNetwork (multi-branch valid conv over embeddings).

Reference math:
  x = emb[tokens]                                   # (1, C=39, H=8192, E=51)
  for k in (1,2,5,10,20):  feat_k = max(relu(conv_valid(x, w_k) + b_k))
  out = softmax(relu(lin_w @ feats + lin_b))        # (8,)

Reformulation:
  Z[t, j] = sum_{c,e} x[c,t,e] * W[(c,e), j]        # 38 cols, one per (branch, tap)
  y_k[h]  = sum_{dh<k} Z[h+dh, off_k + dh]          # shifted-sum via selector matmuls
  feat_k  = relu(max_h y_k[h] + b_k)                # bias/relu folded on host (monotone)

Sharding: H split across 8 cores, 1024 positions each + 19 halo rows computed
redundantly (no collectives).  Each core returns its per-branch partial maxima
[5, 2]; the host reduces across cores and applies the tiny linear+softmax head.

Device pipeline per core (raw bass, explicit semaphores — this walrus build
rejects any instruction carrying more than one semaphore wait, so waits are
emitted as standalone wait_ge ops and each instruction needs at most one):
  Pool : indirect-DMA embedding gather (bf16 rows padded to 64 cols = 128B),
         channel pairs packed [c0|c1] per 128-wide slab.
  PE   : transpose slabs into 3 rotating PSUM banks; accumulate Z (bf16
         matmuls, 20 channel-pair chunks); selector+mask matmuls (f32r).
  DVE  : copy transposed slabs PSUM->SBUF (3 slabs per copy); copy Z PSUM->
         SBUF; final free-dim reduce_max.
  SP   : input DMAs, output DMA.
"""

import numpy as np

VOCAB, EDIM, C, H = 87429, 51, 39, 8192
KS = (1, 2, 5, 10, 20)
OFF = (0, 1, 3, 8, 18)          # column offset of each branch inside the 38 cols
NCOL = 38                       # sum(KS)
EP = 64                         # embedding row padded to 64 cols (bf16 -> 128B)
NCORES = 8
HLOC = H // NCORES              # 1024 output positions per core
HALO = max(KS) - 1              # 19
HW_ = HLOC + HALO               # 1043 Z rows needed per core
G = 9                           # h-groups of 128 per core (1152 slots)
HPAD = G * 128                  # 1152
NP = 20                         # channel pairs (39 real channels + 1 dummy)
NIDX = NP * 2 * G               # 360 index columns
GATHER_GROUPS = (1, 1, 2, 4, 4, 4, 4)   # pairs per indirect-DMA instruction

_CACHE = {}


def _bf16():
    import ml_dtypes
    return np.dtype(ml_dtypes.bfloat16)


def _lint_waits(nc):
    bad = []
    for bb in nc.m.functions[0].blocks:
        for i in bb.instructions:
            si = i.sync_info
            w = [(x.ant_name, x.wait_value) for x in si.on_wait] if si and si.on_wait else []
            if len(w) >= 2:
                bad.append((i.name, type(i).__name__, str(i.engine), w))
    if bad:
        for b in bad:
            print("MULTIWAIT:", b)
        raise RuntimeError(f"{len(bad)} instructions with >1 sync wait")


def _build_bass():
    from contextlib import ExitStack

    import concourse.bass as bass
    from concourse import mybir

    f32 = mybir.dt.float32
    bf16 = mybir.dt.bfloat16
    f32r = mybir.dt.float32r
    i32 = mybir.dt.int32

    nc = bass.Bass(trn_type="TRN2")

    idx_d = nc.dram_tensor("idx", [128, NIDX], i32, kind="ExternalInput")
    emb_d = nc.dram_tensor("emb", [VOCAB, EP], bf16, kind="ExternalInput")
    w_d = nc.dram_tensor("wmat", [128, NP * NCOL], bf16, kind="ExternalInput")
    s_d = nc.dram_tensor("smat", [NCOL, 105], f32, kind="ExternalInput")
    ym_d = nc.dram_tensor("ymask", [5, HLOC], f32, kind="ExternalInput")
    id_d = nc.dram_tensor("ident", [128, 128], bf16, kind="ExternalInput")
    out_d = nc.dram_tensor("out", [5, 2], f32, kind="ExternalOutput")

    # pair q -> gather-group index, and per-group cumulative pair counts
    pair_group = []
    for gi, npair in enumerate(GATHER_GROUPS):
        pair_group += [gi] * npair
    assert len(pair_group) == NP

    with ExitStack() as ctx:
        def sb(name, shape, dt):
            return ctx.enter_context(nc.sbuf_tensor(name, shape, dt))

        def ps(name, shape, dt):
            return ctx.enter_context(nc.psum_tensor(name, shape, dt))

        def mksem(name):
            return ctx.enter_context(nc.semaphore(name))

        idx_sb = sb("idx_sb", [128, NIDX], i32)
        w_sb = sb("w_sb", [128, NP * NCOL], bf16)
        s_sb = sb("s_sb", [NCOL, 105], f32)
        ym_sb = sb("ym_sb", [5, HLOC], f32)
        s_sbr = sb("s_sbr", [NCOL, 105], f32r)
        ym_sbr = sb("ym_sbr", [5, HLOC], f32r)
        id_sb = sb("id_sb", [128, 128], bf16)
        z_sb = sb("z_sb", [NCOL, HW_], f32r)
        out_sb = sb("out_sb", [5, 2], f32)
        pair_all = sb("pair_all", [128, NP * HPAD], bf16)
        xt_all = sb("xt_all", [128, NP * HPAD], bf16)

        z0 = ps("z0", [NCOL, 512], f32)
        z1 = ps("z1", [NCOL, 512], f32)
        z2 = ps("z2", [NCOL, HALO], f32)
        tp = [ps(f"tp{i}", [128, 384], bf16) for i in range(3)]
        yb = [ps("y0", [5, 512], f32), ps("y1", [5, 512], f32)]

        in_sem = mksem("in_sem")
        gsems = [mksem(f"gsem{i}") for i in range(NP)]
        tsem = mksem("tsem")
        dsem = mksem("dsem")
        zsem = mksem("zsem")
        ysem = mksem("ysem")
        rsem = mksem("rsem")
        osem = mksem("osem")

        with nc.Block() as block:

            @block.sync
            def _(sp):
                sp.dma_start(out=idx_sb[:], in_=idx_d[:]).then_inc(in_sem, 16)
                sp.dma_start(out=w_sb[:], in_=w_d[:]).then_inc(in_sem, 16)
                sp.dma_start(out=s_sb[:], in_=s_d[:]).then_inc(in_sem, 16)
                sp.dma_start(out=ym_sb[:], in_=ym_d[:]).then_inc(in_sem, 16)
                sp.dma_start(out=id_sb[:], in_=id_d[:]).then_inc(in_sem, 16)
                sp.wait_ge(rsem, 2)
                sp.dma_start(out=out_d[:], in_=out_sb[:]).then_inc(osem, 16)
                sp.wait_ge(osem, 16)

            @block.gpsimd
            def _(pool):
                pool.wait_ge(in_sem, 80)
                # ucode only honors one index per partition per call, so issue
                # one [128,1]-index gather per (channel, h-group) column.
                for q in range(NP):
                    for j in range(2 * G):
                        col = q * 2 * G + j
                        nc.gpsimd.indirect_dma_start(
                            out=pair_all[:, col * EP:(col + 1) * EP],
                            out_offset=None,
                            in_=emb_d[:],
                            in_offset=bass.IndirectOffsetOnAxis(
                                ap=idx_sb[:, col:col + 1], axis=0
                            ),
                        ).then_inc(gsems[q], 16)

            @block.tensor
            def _(pe):
                pe.wait_ge(in_sem, 80)
                for q in range(NP):
                    pe.wait_ge(gsems[q], 16 * 2 * G)
                    for g in range(G):
                        n = 9 * q + g
                        m, j = divmod(n, 3)
                        nc.tensor.transpose(
                            out=tp[m % 3][:, j * 128:(j + 1) * 128],
                            in_=pair_all[:, q * HPAD + g * 128: q * HPAD + (g + 1) * 128],
                            identity=id_sb[:],
                        ).then_inc(tsem, 1)
                    pe.wait_ge(dsem, 3 * (q + 1))
                    lhsT = w_sb[:, q * NCOL:(q + 1) * NCOL]
                    xq = q * HPAD
                    st = dict(start=(q == 0), stop=(q == NP - 1))
                    nc.tensor.matmul(out=z0[:], lhsT=lhsT,
                                     rhs=xt_all[:, xq: xq + 512], **st)
                    nc.tensor.matmul(out=z1[:], lhsT=lhsT,
                                     rhs=xt_all[:, xq + 512: xq + 1024], **st)
                    mm = nc.tensor.matmul(out=z2[:], lhsT=lhsT,
                                          rhs=xt_all[:, xq + 1024: xq + 1024 + HALO],
                                          **st)
                    if q == NP - 1:
                        mm.then_inc(zsem, 1)
                pe.wait_ge(dsem, 63)
                for b in range(2):
                    for dh in range(20):
                        nc.tensor.matmul(
                            out=yb[b][:],
                            lhsT=s_sbr[:, dh * 5:(dh + 1) * 5],
                            rhs=z_sb[:, b * 512 + dh: b * 512 + dh + 512],
                            start=(dh == 0), stop=False,
                        )
                    nc.tensor.matmul(
                        out=yb[b][:],
                        lhsT=s_sbr[:5, 100:105],
                        rhs=ym_sbr[:, b * 512:(b + 1) * 512],
                        start=False, stop=True,
                    ).then_inc(ysem, 1)

            @block.vector
            def _(dve):
                dve.wait_ge(in_sem, 80)
                nc.vector.tensor_copy(out=s_sbr[:], in_=s_sb[:])
                nc.vector.tensor_copy(out=ym_sbr[:], in_=ym_sb[:])
                for m in range(60):
                    q, j = divmod(m, 3)
                    dve.wait_ge(tsem, 3 * m + 3)
                    nc.vector.tensor_copy(
                        out=xt_all[:, q * HPAD + j * 384: q * HPAD + (j + 1) * 384],
                        in_=tp[m % 3][:],
                    ).then_inc(dsem, 1)
                dve.wait_ge(zsem, 1)
                nc.vector.tensor_copy(out=z_sb[:, 0:512], in_=z0[:]).then_inc(dsem, 1)
                nc.vector.tensor_copy(out=z_sb[:, 512:1024], in_=z1[:]).then_inc(dsem, 1)
                nc.vector.tensor_copy(out=z_sb[:, 1024:HW_], in_=z2[:]).then_inc(dsem, 1)
                for b in range(2):
                    dve.wait_ge(ysem, b + 1)
                    nc.vector.reduce_max(
                        out=out_sb[:, b:b + 1], in_=yb[b][:],
                        axis=mybir.AxisListType.X,
                    ).then_inc(rsem, 1)

    _lint_waits(nc)
    return nc


def _prep_shared(emb, ws):
    bf = _bf16()
    emb_pad = np.zeros((VOCAB, EP), np.float32)
    emb_pad[:, :EDIM] = np.asarray(emb, np.float32)
    emb_pad = emb_pad.astype(bf)

    # W' rows laid out as (c, e<64) so channel pairs align to 128 partitions.
    wfull = np.zeros((NP * 2, EP, NCOL), np.float32)
    for ki, k in enumerate(KS):
        wk = np.asarray(ws[ki], np.float32)[0]          # (C, k, E)
        wfull[:C, :EDIM, OFF[ki]:OFF[ki] + k] = wk.transpose(0, 2, 1)
    wmat = np.ascontiguousarray(
        wfull.reshape(NP, 128, NCOL).transpose(1, 0, 2)
    ).reshape(128, NP * NCOL).astype(bf)

    smat = np.zeros((NCOL, 105), np.float32)
    sm = np.zeros((NCOL, 20, 5), np.float32)
    for ki, k in enumerate(KS):
        for dh in range(k):
            sm[OFF[ki] + dh, dh, ki] = 1.0
    smat[:, :100] = sm.reshape(NCOL, 100)
    smat[:5, 100:105] = np.eye(5, dtype=np.float32)

    ident = np.eye(128, dtype=np.float32).astype(bf)
    return emb_pad, wmat, smat, ident


def _prep_core(tokens_cxh, core):
    s = core * HLOC
    tok = np.zeros((NP * 2, HPAD), np.int32)
    end = min(s + HW_, H)
    tok[:C, :end - s] = tokens_cxh[:, s:end]
    # idx[p, q*18 + g*2 + sub] = tok[2q+sub, g*128+p]
    idx = np.ascontiguousarray(
        tok.reshape(NP, 2, G, 128).transpose(3, 0, 2, 1)
    ).reshape(128, NIDX)

    ym = np.zeros((5, HLOC), np.float32)
    if core == NCORES - 1:
        for ki, k in enumerate(KS):
            if k > 1:
                ym[ki, HLOC + 1 - k:] = -1e30
    return idx, ym


def _run_device(in_maps, trace=False, tmpdir=None):
    from concourse.bass_utils import run_bass_kernel_spmd

    if "nc" not in _CACHE:
        _CACHE["nc"] = _build_bass()
    return run_bass_kernel_spmd(
        _CACHE["nc"], in_maps, core_ids=list(range(NCORES)),
        trace=trace, tmpdir=tmpdir,
    )


def kernel_with_results(tokens, emb, lin_w, lin_b,
                        w0, b0, w1, b1, w2, b2, w3, b3, w4, b4,
                        trace=False, tmpdir=None):
    tokens_cxh = np.asarray(tokens).astype(np.int32).reshape(C, H)
    emb_pad, wmat, smat, ident = _prep_shared(emb, (w0, w1, w2, w3, w4))

    in_maps = []
    for core in range(NCORES):
        idx, ym = _prep_core(tokens_cxh, core)
        in_maps.append({
            "idx": idx, "emb": emb_pad, "wmat": wmat, "smat": smat,
            "ymask": ym, "ident": ident,
        })

    res = _run_device(in_maps, trace=trace, tmpdir=tmpdir)
    partial = np.stack([np.asarray(r["out"]) for r in res.results])  # [8, 5, 2]
    gmax = partial.max(axis=(0, 2)).astype(np.float32)               # [5]

    bs = np.array([np.asarray(b).reshape(-1)[0] for b in (b0, b1, b2, b3, b4)],
                  np.float32)
    feats = np.maximum(gmax + bs, 0.0)
    logits = np.maximum(
        np.asarray(lin_w, np.float32) @ feats + np.asarray(lin_b, np.float32), 0.0
    )
    e = np.exp(logits - logits.max())
    return (e / e.sum()).astype(np.float32), res


def kernel(**inputs):
    out, _ = kernel_with_results(**inputs)
    return out

